# revision 31
# baseline (speedup 1.0000x reference)
"""Trainium2 Bass kernel for GQA attention block (B=2, S=2048, HS=2048, H=16, HKV=4, D=128).

Strategy (8 NeuronCores, SPMD), v2 — batch x kv-head sharding:
  - Core c = (batch b=c//4, kv-head g=c%4): computes q-heads {4g..4g+3} and
    kv-head g for ONLY its batch's 2048 tokens. This removes the kv-projection
    redundancy of head-parallel sharding (each kv head was computed twice) and
    halves per-core x traffic.
  - Fused QKV projection: per contraction tile, one N=512 matmul (4 q heads)
    + one N=256 matmul (k|v). Per-head RMS norm + RoPE in [tok, d] layout
    reading PSUM directly; the norm multiply is folded into the PE transpose
    via diag(1/rms) streaming operands (norm weights and 1/sqrt(D) folded into
    host-precomputed cos/sin tables).
  - Causal flash attention in transposed layout: S^T = K_rope @ Q_rope^T
    ([kv, q]), exp on ScalarE (|scores| <= sqrt(D), no max subtraction),
    diagonal blocks narrowed to the causal triangle. O^T = V^T @ P^T in PSUM.
    Softmax denominators: exp-probs accumulated across kv blocks on VectorE
    into a [128, 512] tile, then ONE ones-matmul per q-chunk (instead of a
    ones-matmul per kv block — saves ~30% of attention PE columns).
  - One 8-rank AllToAll per local q-head redistributes (head, batch) shards ->
    256-token strips of BOTH batches per core; o-projection accumulates the 4
    head-groups in fp32 SBUF across 4 passes, each overlapped with the next
    head's attention.
"""

import sys

sys.path.insert(0, "/opt/trn_rl_repo")

import numpy as np
import ml_dtypes

BF16 = ml_dtypes.bfloat16

B, H, HKV, D = 2, 16, 4, 128
EPS = 1e-6
P = 128
N_CORES = 8
NQ = 4              # q heads per core


def build(S=2048, HS=2048):
    """Build + compile the SPMD graph. Returns the Bacc module."""
    import concourse.bacc as bacc
    import concourse.tile as tile
    import concourse.mybir as mybir

    dt = mybir.dt
    f32 = dt.float32
    bf16 = dt.bfloat16
    AF = mybir.ActivationFunctionType
    ALU = mybir.AluOpType

    T = S // P          # tok tiles for this core's batch (16)
    KT = HS // P        # contraction tiles for qkv projection (16)
    KO = (H * D) // P   # contraction tiles for o projection (16)
    CW = S // 4         # q-chunk width (512)
    CB = CW // P        # kv blocks per chunk step (4)
    SW = 256            # output strip width per batch
    OCH = HS // 512     # output column chunks (4)

    nc = bacc.Bacc("TRN2", target_bir_lowering=False, debug=False,
                   enable_asserts=True, num_devices=N_CORES)

    xT = nc.dram_tensor("xT", [T, P, HS], bf16, kind="ExternalInput")
    wqkvT = nc.dram_tensor("wqkvT", [P, KT * 768], bf16, kind="ExternalInput")
    woT = nc.dram_tensor("woT", [P, KO * HS], bf16, kind="ExternalInput")
    cosq_d = nc.dram_tensor("cosq", [P, T * D], bf16, kind="ExternalInput")
    sinq_d = nc.dram_tensor("sinq", [P, T * D], bf16, kind="ExternalInput")
    cosk_d = nc.dram_tensor("cosk", [P, T * D], bf16, kind="ExternalInput")
    sink_d = nc.dram_tensor("sink", [P, T * D], bf16, kind="ExternalInput")
    tri_d = nc.dram_tensor("tri", [P, P], bf16, kind="ExternalInput")
    onesq_d = nc.dram_tensor("onesq", [P, P], bf16, kind="ExternalInput")
    ident_d = nc.dram_tensor("ident", [P, P], bf16, kind="ExternalInput")
    out_d = nc.dram_tensor("out", [2 * SW, HS], bf16, kind="ExternalOutput")

    with tile.TileContext(nc) as tc:
        with tc.tile_pool(name="const", bufs=1) as cpool, \
             tc.tile_pool(name="qkv", bufs=1) as qkvpool, \
             tc.tile_pool(name="dram", bufs=1, space="DRAM") as dpool:

            cosq_sb = cpool.tile([P, T, D], bf16, name="cosq_sb")
            sinq_sb = cpool.tile([P, T, D], bf16, name="sinq_sb")
            cosk_sb = cpool.tile([P, T, D], bf16, name="cosk_sb")
            sink_sb = cpool.tile([P, T, D], bf16, name="sink_sb")
            tri_sb = cpool.tile([P, P], bf16, name="tri_sb")
            onesq_sb = cpool.tile([P, P], bf16, name="onesq_sb")
            ident_sb = cpool.tile([P, P], bf16, name="ident_sb")
            eps_sb = cpool.tile([P, 1], f32, name="eps_sb")
            nc.gpsimd.memset(eps_sb[:], EPS)
            scr_sb = cpool.tile([P, 1], f32, name="scr_sb")
            # prewarm the ACT Exp table so its load isn't on the critical path
            # at the stage-A -> attention boundary
            nc.scalar.activation(scr_sb, eps_sb, AF.Exp)

            qT = qkvpool.tile([P, NQ, S], bf16, name="qT")
            kT = qkvpool.tile([P, S], bf16, name="kT")
            v_sb = qkvpool.tile([P, T, D], bf16, name="v_sb")

            a2a_in = [dpool.tile([8 * P, SW], bf16, name=f"a2a_in{h}")
                      for h in range(NQ)]
            a2a_out = [dpool.tile([8 * P, SW], bf16, name=f"a2a_out{h}")
                       for h in range(NQ)]

            # ---------------- stage 1+2: QKV projection, RMS norm, RoPE, transpose
            with tc.tile_pool(name="wq", bufs=1) as wpool, \
                 tc.tile_pool(name="s12", bufs=2) as s12, \
                 tc.tile_pool(name="xin", bufs=6) as xin, \
                 tc.tile_pool(name="ps12", bufs=2, space="PSUM") as ps12:
                wqkv_sb = wpool.tile([P, KT, 768], bf16, name="wqkv_sb")
                wqkv_src = wqkvT.ap().rearrange("p (k f) -> p k f", k=KT)
                xms = {}

                def load_xm(m, split=1):
                    t_ = xin.tile([P, KT, P], bf16, tag="xm", name=f"xm{m}")
                    kk = KT // split
                    src = xT.ap()[m].rearrange("p (k t) -> p k t", k=KT)
                    for s4 in range(0, KT, kk):
                        nc.sync.dma_start(t_[:, s4:s4 + kk, :],
                                          src[:, s4:s4 + kk, :])
                    xms[m] = t_

                # startup-critical DMA order: first wqkv slice + first x tile in
                # 4 parallel pieces so matmuls start ASAP.
                nc.sync.dma_start(wqkv_sb[:, 0, :], wqkv_src[:, 0, :])
                load_xm(0, split=4)
                for k in range(1, 4):
                    nc.sync.dma_start(wqkv_sb[:, k, :], wqkv_src[:, k, :])
                # tiny dummy AllToAll: absorbs the ~11us first-collective mesh
                # warmup on the CC core while the PE is busy with stage 1
                dummy_sb = cpool.tile([8, 64], bf16, name="dummy_sb")
                nc.gpsimd.memset(dummy_sb[:], 0.0)
                dummy_in = dpool.tile([8, 64], bf16, name="dummy_in")
                dummy_out = dpool.tile([8, 64], bf16, name="dummy_out")
                nc.sync.dma_start(dummy_in[:], dummy_sb[:])
                nc.gpsimd.collective_compute(
                    "AllToAll", mybir.AluOpType.bypass,
                    ins=[dummy_in[:].opt()], outs=[dummy_out[:].opt()],
                    replica_groups=[list(range(N_CORES))],
                )
                load_xm(1, split=2)
                for k in range(4, KT):
                    nc.sync.dma_start(wqkv_sb[:, k, :], wqkv_src[:, k, :])
                nc.sync.dma_start(cosq_sb[:],
                                  cosq_d.ap().rearrange("p (t d) -> p t d", t=T))
                nc.sync.dma_start(sinq_sb[:],
                                  sinq_d.ap().rearrange("p (t d) -> p t d", t=T))
                nc.sync.dma_start(ident_sb[:], ident_d.ap())
                load_xm(2)
                nc.sync.dma_start(cosk_sb[:],
                                  cosk_d.ap().rearrange("p (t d) -> p t d", t=T))
                nc.sync.dma_start(sink_sb[:],
                                  sink_d.ap().rearrange("p (t d) -> p t d", t=T))
                load_xm(3)
                load_xm(4)
                nc.sync.dma_start(tri_sb[:], tri_d.ap())
                nc.sync.dma_start(onesq_sb[:], onesq_d.ap())

                # junk matmuls on a memset tile: warms the PE HAM clock gate
                # (~3.4us of sustained activity -> 2.4 GHz) while the first
                # real input tiles are still in flight on DMA.
                warm_sb = cpool.tile([P, 512], bf16, name="warm_sb")
                nc.gpsimd.memset(warm_sb[:], 0.0)
                for i in range(8):
                    wp = ps12.tile([P, 512], f32, tag="psq")
                    nc.tensor.matmul(wp, warm_sb[:, 0:P], warm_sb,
                                     start=True, stop=True)

                pend = None  # previous tile's (ro, rok, diagr, col): transposes
                # are deferred one iteration so the norm/rope chain of tile m
                # overlaps the projection matmuls of tile m+1 (keeps PE dense).

                def emit_transposes(p):
                    ro_, rok_, diagr_, col_ = p
                    for idx in range(NQ + 1):
                        srct = ro_[:, idx * P:(idx + 1) * P] if idx < NQ else rok_
                        dst = qT[:, idx, col_:col_ + P] if idx < NQ \
                            else kT[:, col_:col_ + P]
                        tp = ps12.tile([P, P], f32, tag="tp", bufs=3)
                        nc.tensor.matmul(tp, srct, diagr_[:, idx, :],
                                         start=True, stop=True)
                        nc.scalar.copy(dst, tp)

                for m in range(T):
                    if m + 5 < T:
                        load_xm(m + 5)
                    xm = xms.pop(m)
                    ps_q = ps12.tile([P, 512], f32, tag="psq")
                    ps_kv = ps12.tile([P, 256], f32, tag="pskv")
                    for k in range(KT):
                        nc.tensor.matmul(ps_q, xm[:, k, :], wqkv_sb[:, k, 0:512],
                                         start=(k == 0), stop=(k == KT - 1))
                    for k in range(KT):
                        nc.tensor.matmul(ps_kv, xm[:, k, :], wqkv_sb[:, k, 512:768],
                                         start=(k == 0), stop=(k == KT - 1))
                    # V: plain copy to [tok, d] layout (VectorE; reads PSUM)
                    nc.vector.tensor_copy(v_sb[:, m, :], ps_kv[:, 128:256])
                    col = P * m
                    # squared-sums for q0..q3|k read straight from PSUM (ScalarE)
                    sq = s12.tile([P, P], bf16, tag="sq")
                    ssum = s12.tile([P, NQ + 1], f32, tag="ssum")
                    for idx in range(NQ):
                        nc.scalar.activation(sq, ps_q[:, idx * P:(idx + 1) * P],
                                             AF.Square,
                                             accum_out=ssum[:, idx:idx + 1])
                    nc.scalar.activation(sq, ps_kv[:, 0:P], AF.Square,
                                         accum_out=ssum[:, NQ:NQ + 1])
                    rms = s12.tile([P, NQ + 1], f32, tag="rms")
                    nc.scalar.activation(rms, ssum, AF.Sqrt,
                                         bias=eps_sb[:], scale=1.0 / D)
                    rinv = s12.tile([P, NQ + 1], f32, tag="rinv")
                    nc.vector.reciprocal_approx_fast(rinv, rms)
                    # diag(1/rms) per head: folded into the transpose matmuls
                    diagr = s12.tile([P, NQ + 1, P], bf16, tag="diagr")
                    for idx in range(NQ + 1):
                        nc.vector.tensor_scalar_mul(diagr[:, idx, :], ident_sb,
                                                    rinv[:, idx:idx + 1])
                    # RoPE on raw q straight from PSUM; q heads batched via
                    # broadcast tables. (rope commutes with the per-token norm
                    # scalar, which is applied by the diag-transpose below.)
                    ro = s12.tile([P, NQ * P], bf16, tag="ro")
                    ro_v = ro.rearrange("p (h d) -> p h d", h=NQ)
                    ps_q_h = ps_q[:].rearrange("p (h d) -> p h d", h=NQ)
                    cos_b = cosq_sb[:, m, None, :].to_broadcast((P, NQ, D))
                    nc.vector.tensor_tensor(ro_v, ps_q_h, cos_b, ALU.mult)
                    rh = s12.tile([P, NQ * P], bf16, tag="rh")
                    rh_v = rh.rearrange("p (h x d) -> p h x d", h=NQ, x=2)
                    ps_q_x = ps_q[:].rearrange("p (h x d) -> p h x d", h=NQ, x=2)
                    sinq_m = sinq_sb[:, m, :].rearrange("p (x d) -> p x d", x=2)
                    sinA = sinq_m[:, None, 0, :].to_broadcast((P, NQ, 64))
                    sinB = sinq_m[:, None, 1, :].to_broadcast((P, NQ, 64))
                    nc.vector.tensor_tensor(rh_v[:, :, 0, :], ps_q_x[:, :, 1, :],
                                            sinA, ALU.mult)
                    nc.vector.tensor_tensor(rh_v[:, :, 1, :], ps_q_x[:, :, 0, :],
                                            sinB, ALU.mult)
                    nc.vector.tensor_tensor(ro, ro, rh, ALU.add)
                    # RoPE for k
                    rok = s12.tile([P, P], bf16, tag="rok")
                    nc.vector.tensor_tensor(rok, ps_kv[:, 0:P],
                                            cosk_sb[:, m, :], ALU.mult)
                    rhk = s12.tile([P, P], bf16, tag="rhk")
                    nc.vector.tensor_tensor(rhk[:, 0:64], ps_kv[:, 64:P],
                                            sink_sb[:, m, 0:64], ALU.mult)
                    nc.vector.tensor_tensor(rhk[:, 64:128], ps_kv[:, 0:64],
                                            sink_sb[:, m, 64:128], ALU.mult)
                    nc.vector.tensor_tensor(rok, rok, rhk, ALU.add)
                    if pend is not None:
                        emit_transposes(pend)
                    pend = (ro, rok, diagr, col)
                emit_transposes(pend)

            # prefetch o-projection weights during attention
            wo_sb, _wo_free = tc.tile([P, KO, HS], bf16, name="wo_sb")
            wo_src = woT.ap().rearrange("p (k f) -> p k f", k=KO)
            for k4 in range(0, KO, 4):
                nc.sync.dma_start(wo_sb[:, k4:k4 + 4, :], wo_src[:, k4:k4 + 4, :])

            # ---------------- stage 3+4: causal attention head-major; each
            # head's AllToAll overlaps the next head's compute, and its o-proj
            # pass is interleaved INTO the next head's attention blocks so the
            # PE fills the ScalarE-exp stalls (exp ~580ns/block > PE 526ns).
            acc_ctx = tc.tile_pool(name="acc", bufs=1)
            accp = acc_ctx.__enter__()
            accs = {}
            with tc.tile_pool(name="s3", bufs=6) as s3, \
                 tc.tile_pool(name="s3b", bufs=2) as s3b, \
                 tc.tile_pool(name="s4", bufs=2) as s4, \
                 tc.tile_pool(name="ps3", bufs=1, space="PSUM") as ps3, \
                 tc.tile_pool(name="ps4", bufs=1, space="PSUM") as ps4:
                a_sbs = {}

                def make_groups(h):
                    # 16 closures, each = one o-proj output tile of pass h:
                    # 4 PE matmuls (one per kv-core of the head-group) + an
                    # accumulate into the fp32 SBUF accs.
                    groups = []
                    for st in range(2):       # batch strip
                        for t2 in range(2):   # 128-tok tile within strip
                            for oc in range(OCH):
                                def g(st=st, t2=t2, oc=oc, h=h):
                                    ps_o = ps4.tile([P, 512], f32, tag="op",
                                                    bufs=3)
                                    a_sb = a_sbs[h]
                                    for gp in range(4):
                                        nc.tensor.matmul(
                                            ps_o,
                                            a_sb[:, 4 * st + gp,
                                                 P * t2:P * (t2 + 1)],
                                            wo_sb[:, 4 * gp + h,
                                                  512 * oc:512 * (oc + 1)],
                                            start=(gp == 0), stop=(gp == 3))
                                    key = (st, t2, oc)
                                    if h == 0:
                                        acc = accp.tile([P, 512], f32,
                                                        name=f"acc{st}{t2}{oc}")
                                        nc.scalar.copy(acc, ps_o)
                                        accs[key] = acc
                                    elif h < NQ - 1:
                                        nc.vector.tensor_tensor(
                                            accs[key], ps_o, accs[key], ALU.add)
                                    else:
                                        # bufs=4: the strided out-DMAs take
                                        # ~1.5us each; with 2 bufs the DVE add
                                        # chain throttles to DMA pace
                                        osb = s4.tile([P, 512], bf16,
                                                      tag="osb", bufs=4)
                                        nc.vector.tensor_tensor(
                                            osb, ps_o, accs[key], ALU.add)
                                        r0 = SW * st + P * t2
                                        nc.sync.dma_start(
                                            out_d.ap()[r0:r0 + P,
                                                       512 * oc:512 * (oc + 1)],
                                            osb)
                                groups.append(g)
                    return groups

                pend_groups = []  # (pass_id, closure) o-proj groups, FIFO
                for h in range(NQ):
                    # on the last head interleave nothing: its wall is the
                    # ScalarE exp floor either way, so all 32 held groups
                    # cover the final AllToAll (trigger lags the last block
                    # by ~6us, mesh+gather is another ~17us)
                    head_cap = [0 if h == NQ - 1 else 99]
                    pend_fin = None  # previous chunk's softmax finalize
                    for c in (3, 2, 1, 0):
                        qv = qT[:, h, CW * c: CW * (c + 1)]
                        nb = (c + 1) * CB
                        nd = nb - CB  # non-diagonal blocks (full width)
                        o_ps = ps3.tile([P, CW], f32, tag="o", bufs=2)
                        # exp-prob accumulator for the softmax denominator
                        pacc = s3b.tile([P, CW], bf16, tag="pacc")

                        def emit_acc(kb, pT, off, nb=nb, o_ps=o_ps):
                            # O accumulation for block kb; deferred one block so
                            # the next score matmul hides the exp latency.
                            nc.tensor.matmul(o_ps[:, off:],
                                             v_sb[:, kb, :], pT[:, off:],
                                             start=(kb == 0),
                                             stop=(kb == nb - 1))

                        pend = []
                        for kb in range(nb):
                            j = kb - nd  # diagonal index, >= 0 for diag blocks
                            off = P * j if j >= 0 else 0
                            s_ps = ps3.tile([P, CW], f32, tag="s", bufs=3)
                            nc.tensor.matmul(
                                s_ps[:, off:],
                                kT[:, P * kb: P * (kb + 1)],
                                qv[:, off:], start=True, stop=True)
                            if kb == 0:
                                pT = pacc
                            else:
                                pT = s3.tile([P, CW], bf16, tag="pT")
                            nc.scalar.activation(pT[:, off:], s_ps[:, off:],
                                                 AF.Exp)
                            if j >= 0:
                                # causal triangle on the first P columns of
                                # this diagonal block
                                nc.vector.tensor_tensor(
                                    pT[:, off:off + P], pT[:, off:off + P],
                                    tri_sb, ALU.mult)
                            # O-matmuls deferred 2 blocks so the PE never rides
                            # the exp frontier; block 0 only 1 block (its probs
                            # live in pacc, which block 1's add overwrites)
                            while pend and (pend[0][0] == 0 or len(pend) >= 2):
                                emit_acc(*pend.pop(0))
                            if kb > 0:
                                nc.vector.tensor_tensor(pacc[:, off:],
                                                        pacc[:, off:],
                                                        pT[:, off:], ALU.add)
                            pend.append((kb, pT, off))
                            if kb == 1 and pend_fin is not None:
                                # previous chunk's finalize, deferred two
                                # blocks so its DVE add-chain has drained
                                pend_fin()
                                pend_fin = None
                            # interleave pending o-proj groups into attention
                            # blocks to fill the ScalarE-exp stalls. AllToAll
                            # end-to-end latency varies 14-25us with inter-core
                            # skew, so a pass's groups are only safe TWO heads
                            # later (p <= h-2) — a too-early group hard-stalls
                            # the whole in-order PE stream. Pass NQ-2 is
                            # flushed across the final AllToAll instead, and
                            # pass NQ-1 is the tail.
                            take = (1 if (kb % 2 == 0 and c in (1, 2)) else
                                    2 if c == 0 else 0)
                            take = min(take, head_cap[0])
                            safe = h - 2
                            while take and pend_groups and \
                                    pend_groups[0][0] <= safe:
                                pend_groups.pop(0)[1]()
                                take -= 1
                                head_cap[0] -= 1
                        for p_ in pend:
                            emit_acc(*p_)

                        def finalize(c=c, pacc=pacc, o_ps=o_ps, h=h):
                            # single denominator matmul per chunk
                            sum_ps = ps3.tile([P, CW], f32, tag="s", bufs=3)
                            nc.tensor.matmul(sum_ps, onesq_sb, pacc,
                                             start=True, stop=True)
                            rec = s3b.tile([P, CW], f32, tag="rec")
                            nc.vector.reciprocal_approx_fast(rec, sum_ps)
                            o_sb = s3b.tile([P, CW], bf16, tag="o_sb")
                            nc.vector.tensor_tensor(o_sb, o_ps, rec, ALU.mult)
                            # two 256-token strips -> dest ranks 2c and 2c+1,
                            # fused into one DMA (dest rows are contiguous)
                            nc.sync.dma_start(
                                a2a_in[h][P * 2 * c:P * (2 * c + 2), :]
                                .rearrange("(blk p) t -> p blk t", p=P),
                                o_sb.rearrange("p (blk t) -> p blk t", blk=2))

                        if c > 0:
                            pend_fin = finalize
                        else:
                            finalize()
                    if h == NQ - 1:
                        # flush held-back groups: they keep the PE busy (and
                        # the HAM clock warm) across the final AllToAll
                        for _, g in pend_groups:
                            g()
                        pend_groups = []
                    nc.gpsimd.collective_compute(
                        "AllToAll", mybir.AluOpType.bypass,
                        ins=[a2a_in[h][:].opt()],
                        outs=[a2a_out[h][:].opt()],
                        replica_groups=[list(range(N_CORES))],
                    )
                    a_sb = s3.tile([P, 8, SW], bf16, tag="asb", bufs=2)
                    a2a_view = a2a_out[h][:].rearrange("(g p) t -> p g t", p=P)
                    for g8 in range(8):
                        nc.sync.dma_start(a_sb[:, g8, :], a2a_view[:, g8, :])
                    a_sbs[h] = a_sb
                    pend_groups.extend((h, g) for g in make_groups(h))
                # tail: last head's o-proj pass
                for _, g in pend_groups:
                    g()
            acc_ctx.__exit__(None, None, None)
            _wo_free()

    nc.compile()
    return nc


def shard_inputs(inputs, S=2048, HS=2048):
    """Full problem inputs -> list of 8 per-core in_maps (host-side prep)."""
    x = np.asarray(inputs["x"], np.float32)
    cos = np.asarray(inputs["cos"], np.float32)
    sin = np.asarray(inputs["sin"], np.float32)
    wq = np.asarray(inputs["wq"], np.float32)
    wk = np.asarray(inputs["wk"], np.float32)
    wv = np.asarray(inputs["wv"], np.float32)
    wo = np.asarray(inputs["wo"], np.float32)
    qw = np.asarray(inputs["q_norm_w"], np.float32)
    kw = np.asarray(inputs["k_norm_w"], np.float32)

    T = S // P
    M = 2 * T
    KT = HS // P

    xT_t = np.ascontiguousarray(
        x.reshape(M, P, KT, P).transpose(0, 3, 2, 1).reshape(M, P, HS)).astype(BF16)

    sgn = np.concatenate([-np.ones(64, np.float32), np.ones(64, np.float32)])
    scale = 1.0 / np.sqrt(D)

    def tile_p(a):
        # [(n*P), inner] row-major -> [P, n*inner] partition-major
        n = a.shape[0] // P
        return np.ascontiguousarray(
            a.reshape(n, P, a.shape[1]).transpose(1, 0, 2).reshape(P, -1))

    def fold(w, s):
        w_rot = np.concatenate([w[64:], w[:64]])
        c = tile_p((cos * w[None, :] * s).astype(np.float32)).astype(BF16)
        sn = tile_p((sin * (w_rot * sgn)[None, :] * s).astype(np.float32)).astype(BF16)
        return c, sn

    cosq, sinq = fold(qw, scale)
    cosk, sink = fold(kw, 1.0)

    r = np.arange(P)[:, None]
    t = np.arange(P)[None, :]
    tri = (r <= t).astype(BF16)

    onesq = np.ones((P, P), BF16)
    ident = np.eye(P, dtype=np.float32).astype(BF16)
    woT = tile_p(np.ascontiguousarray(wo.T)).astype(BF16)

    in_maps = []
    for c in range(N_CORES):
        b, g = c // 4, c % 4
        wq_c = wq[4 * g * D:(4 * g + 4) * D]       # [512, HS]
        wk_c = wk[g * D:(g + 1) * D]               # [128, HS]
        wv_c = wv[g * D:(g + 1) * D]               # [128, HS]
        wqkv = np.concatenate([wq_c, wk_c, wv_c], axis=0)  # [768, HS]
        wqkvT = tile_p(np.ascontiguousarray(wqkv.T)).astype(BF16)  # [P, KT*768]
        in_maps.append({
            "xT": xT_t[b * T:(b + 1) * T], "wqkvT": wqkvT, "woT": woT,
            "cosq": cosq, "sinq": sinq, "cosk": cosk, "sink": sink,
            "tri": tri, "onesq": onesq, "ident": ident,
        })
    return in_maps


def assemble(outs, S=2048, HS=2048):
    """Per-core strip outputs -> full [B, S, HS] output."""
    SW = 256
    full = np.empty((B, S, HS), np.float32)
    for c in range(N_CORES):
        o = np.asarray(outs[c], dtype=np.float32)
        full[0, c * SW:(c + 1) * SW, :] = o[0:SW]
        full[1, c * SW:(c + 1) * SW, :] = o[SW:2 * SW]
    return full


_CACHE = {}


def _get_compiled(S=2048, HS=2048):
    key = (S, HS)
    if key not in _CACHE:
        _CACHE[key] = build(S, HS)
    return _CACHE[key]


def _ensure_ntff_hook():
    """The image's antenv lacks axon_hooks; synthesize it so trace=True works."""
    import types
    try:
        from antenv.axon_hooks import get_axon_ntff_profile_hook  # noqa: F401
        return
    except ImportError:
        pass
    import antenv
    from trn_agent_boot.trn_boot import _ntff_profile_via_ctypes
    mod = types.ModuleType("antenv.axon_hooks")
    mod._hook = _ntff_profile_via_ctypes("/opt/axon/libaxon_pjrt.so")
    mod.set_axon_ntff_profile_hook = lambda h: setattr(mod, "_hook", h)
    mod.get_axon_ntff_profile_hook = lambda: mod._hook
    sys.modules["antenv.axon_hooks"] = mod
    antenv.axon_hooks = mod


def run(inputs, S=2048, HS=2048, trace=False, tmpdir=None):
    import concourse.bass_utils as bu
    if trace:
        _ensure_ntff_hook()
        bu.upload_artifacts = lambda d: ""  # no artifact bucket in this container
    nc = _get_compiled(S, HS)
    in_maps = shard_inputs(inputs, S, HS)
    res = bu.run_bass_kernel_spmd(nc, in_maps, core_ids=list(range(N_CORES)),
                                  trace=trace, tmpdir=tmpdir)
    out = assemble([r["out"] for r in res.results], S, HS)
    return out, res.exec_time_ns


def kernel(**inputs):
    out, _ = run(inputs)
    return out


# revision 32
# speedup vs baseline: 1.0478x; 1.0478x over previous
"""Trainium2 Bass kernel for GQA attention block (B=2, S=2048, HS=2048, H=16, HKV=4, D=128).

Strategy (8 NeuronCores, SPMD), v2 — batch x kv-head sharding:
  - Core c = (batch b=c//4, kv-head g=c%4): computes q-heads {4g..4g+3} and
    kv-head g for ONLY its batch's 2048 tokens. This removes the kv-projection
    redundancy of head-parallel sharding (each kv head was computed twice) and
    halves per-core x traffic.
  - Fused QKV projection: per contraction tile, one N=512 matmul (4 q heads)
    + one N=256 matmul (k|v). Per-head RMS norm + RoPE in [tok, d] layout
    reading PSUM directly; the norm multiply is folded into the PE transpose
    via diag(1/rms) streaming operands (norm weights and 1/sqrt(D) folded into
    host-precomputed cos/sin tables).
  - Causal flash attention in transposed layout: S^T = K_rope @ Q_rope^T
    ([kv, q]), exp on ScalarE (|scores| <= sqrt(D), no max subtraction),
    diagonal blocks narrowed to the causal triangle. O^T = V^T @ P^T in PSUM.
    Softmax denominators: exp-probs accumulated across kv blocks on VectorE
    into a [128, 512] tile, then ONE ones-matmul per q-chunk (instead of a
    ones-matmul per kv block — saves ~30% of attention PE columns).
  - One 8-rank AllToAll per local q-head redistributes (head, batch) shards ->
    256-token strips of BOTH batches per core; o-projection accumulates the 4
    head-groups in fp32 SBUF across 4 passes, each overlapped with the next
    head's attention.
"""

import sys

sys.path.insert(0, "/opt/trn_rl_repo")

import numpy as np
import ml_dtypes

BF16 = ml_dtypes.bfloat16

B, H, HKV, D = 2, 16, 4, 128
EPS = 1e-6
P = 128
N_CORES = 8
NQ = 4              # q heads per core


def build(S=2048, HS=2048):
    """Build + compile the SPMD graph. Returns the Bacc module."""
    import concourse.bacc as bacc
    import concourse.tile as tile
    import concourse.mybir as mybir

    dt = mybir.dt
    f32 = dt.float32
    bf16 = dt.bfloat16
    AF = mybir.ActivationFunctionType
    ALU = mybir.AluOpType

    T = S // P          # tok tiles for this core's batch (16)
    KT = HS // P        # contraction tiles for qkv projection (16)
    KO = (H * D) // P   # contraction tiles for o projection (16)
    CW = S // 4         # q-chunk width (512)
    CB = CW // P        # kv blocks per chunk step (4)
    SW = 256            # output strip width per batch
    OCH = HS // 512     # output column chunks (4)

    nc = bacc.Bacc("TRN2", target_bir_lowering=False, debug=False,
                   enable_asserts=True, num_devices=N_CORES)

    xT = nc.dram_tensor("xT", [T, P, HS], bf16, kind="ExternalInput")
    wqkvT = nc.dram_tensor("wqkvT", [P, KT * 768], bf16, kind="ExternalInput")
    woT = nc.dram_tensor("woT", [P, KO * HS], bf16, kind="ExternalInput")
    cosq_d = nc.dram_tensor("cosq", [P, T * D], bf16, kind="ExternalInput")
    sinq_d = nc.dram_tensor("sinq", [P, T * D], bf16, kind="ExternalInput")
    cosk_d = nc.dram_tensor("cosk", [P, T * D], bf16, kind="ExternalInput")
    sink_d = nc.dram_tensor("sink", [P, T * D], bf16, kind="ExternalInput")
    tri_d = nc.dram_tensor("tri", [P, P], bf16, kind="ExternalInput")
    onesq_d = nc.dram_tensor("onesq", [P, P], bf16, kind="ExternalInput")
    ident_d = nc.dram_tensor("ident", [P, P], bf16, kind="ExternalInput")
    out_d = nc.dram_tensor("out", [2 * SW, HS], bf16, kind="ExternalOutput")

    with tile.TileContext(nc) as tc:
        with tc.tile_pool(name="const", bufs=1) as cpool, \
             tc.tile_pool(name="qkv", bufs=1) as qkvpool, \
             tc.tile_pool(name="dram", bufs=1, space="DRAM") as dpool:

            cosq_sb = cpool.tile([P, T, D], bf16, name="cosq_sb")
            sinq_sb = cpool.tile([P, T, D], bf16, name="sinq_sb")
            cosk_sb = cpool.tile([P, T, D], bf16, name="cosk_sb")
            sink_sb = cpool.tile([P, T, D], bf16, name="sink_sb")
            tri_sb = cpool.tile([P, P], bf16, name="tri_sb")
            onesq_sb = cpool.tile([P, P], bf16, name="onesq_sb")
            ident_sb = cpool.tile([P, P], bf16, name="ident_sb")
            eps_sb = cpool.tile([P, 1], f32, name="eps_sb")
            nc.gpsimd.memset(eps_sb[:], EPS)
            scr_sb = cpool.tile([P, 1], f32, name="scr_sb")
            # prewarm the ACT Exp table so its load isn't on the critical path
            # at the stage-A -> attention boundary
            nc.scalar.activation(scr_sb, eps_sb, AF.Exp)

            qT = qkvpool.tile([P, NQ, S], bf16, name="qT")
            kT = qkvpool.tile([P, S], bf16, name="kT")
            v_sb = qkvpool.tile([P, T, D], bf16, name="v_sb")

            a2a_in = [dpool.tile([8 * P, SW], bf16, name=f"a2a_in{h}")
                      for h in range(NQ)]
            a2a_out = [dpool.tile([8 * P, SW], bf16, name=f"a2a_out{h}")
                       for h in range(NQ)]

            # ---------------- stage 1+2: QKV projection, RMS norm, RoPE, transpose
            with tc.tile_pool(name="wq", bufs=1) as wpool, \
                 tc.tile_pool(name="s12", bufs=2) as s12, \
                 tc.tile_pool(name="xin", bufs=6) as xin, \
                 tc.tile_pool(name="ps12", bufs=2, space="PSUM") as ps12:
                wqkv_sb = wpool.tile([P, KT, 768], bf16, name="wqkv_sb")
                wqkv_src = wqkvT.ap().rearrange("p (k f) -> p k f", k=KT)
                xms = {}

                def load_xm(m, split=1):
                    t_ = xin.tile([P, KT, P], bf16, tag="xm", name=f"xm{m}")
                    kk = KT // split
                    src = xT.ap()[m].rearrange("p (k t) -> p k t", k=KT)
                    for s4 in range(0, KT, kk):
                        nc.sync.dma_start(t_[:, s4:s4 + kk, :],
                                          src[:, s4:s4 + kk, :])
                    xms[m] = t_

                # startup-critical DMA order: first wqkv slice + first x tile in
                # 4 parallel pieces so matmuls start ASAP.
                nc.sync.dma_start(wqkv_sb[:, 0, :], wqkv_src[:, 0, :])
                load_xm(0, split=4)
                for k in range(1, 4):
                    nc.sync.dma_start(wqkv_sb[:, k, :], wqkv_src[:, k, :])
                # tiny dummy AllToAll: absorbs the ~11us first-collective mesh
                # warmup on the CC core while the PE is busy with stage 1
                dummy_sb = cpool.tile([8, 64], bf16, name="dummy_sb")
                nc.gpsimd.memset(dummy_sb[:], 0.0)
                dummy_in = dpool.tile([8, 64], bf16, name="dummy_in")
                dummy_out = dpool.tile([8, 64], bf16, name="dummy_out")
                nc.sync.dma_start(dummy_in[:], dummy_sb[:])
                nc.gpsimd.collective_compute(
                    "AllToAll", mybir.AluOpType.bypass,
                    ins=[dummy_in[:].opt()], outs=[dummy_out[:].opt()],
                    replica_groups=[list(range(N_CORES))],
                )
                load_xm(1, split=2)
                for k in range(4, KT):
                    nc.sync.dma_start(wqkv_sb[:, k, :], wqkv_src[:, k, :])
                nc.sync.dma_start(cosq_sb[:],
                                  cosq_d.ap().rearrange("p (t d) -> p t d", t=T))
                nc.sync.dma_start(sinq_sb[:],
                                  sinq_d.ap().rearrange("p (t d) -> p t d", t=T))
                nc.sync.dma_start(ident_sb[:], ident_d.ap())
                load_xm(2)
                nc.sync.dma_start(cosk_sb[:],
                                  cosk_d.ap().rearrange("p (t d) -> p t d", t=T))
                nc.sync.dma_start(sink_sb[:],
                                  sink_d.ap().rearrange("p (t d) -> p t d", t=T))
                load_xm(3)
                load_xm(4)
                nc.sync.dma_start(tri_sb[:], tri_d.ap())
                nc.sync.dma_start(onesq_sb[:], onesq_d.ap())

                # junk matmuls on a memset tile: warms the PE HAM clock gate
                # (~3.4us of sustained activity -> 2.4 GHz) while the first
                # real input tiles are still in flight on DMA.
                warm_sb = cpool.tile([P, 512], bf16, name="warm_sb")
                nc.gpsimd.memset(warm_sb[:], 0.0)
                for i in range(8):
                    wp = ps12.tile([P, 512], f32, tag="psq")
                    nc.tensor.matmul(wp, warm_sb[:, 0:P], warm_sb,
                                     start=True, stop=True)

                pend = None  # previous tile's (ro, rok, diagr, col): transposes
                # are deferred one iteration so the norm/rope chain of tile m
                # overlaps the projection matmuls of tile m+1 (keeps PE dense).

                def emit_transposes(p):
                    ro_, rok_, diagr_, col_ = p
                    for idx in range(NQ + 1):
                        srct = ro_[:, idx * P:(idx + 1) * P] if idx < NQ else rok_
                        dst = qT[:, idx, col_:col_ + P] if idx < NQ \
                            else kT[:, col_:col_ + P]
                        tp = ps12.tile([P, P], f32, tag="tp", bufs=3)
                        nc.tensor.matmul(tp, srct, diagr_[:, idx, :],
                                         start=True, stop=True)
                        nc.scalar.copy(dst, tp)

                for m in range(T):
                    if m + 5 < T:
                        load_xm(m + 5)
                    xm = xms.pop(m)
                    ps_q = ps12.tile([P, 512], f32, tag="psq")
                    ps_kv = ps12.tile([P, 256], f32, tag="pskv")
                    for k in range(KT):
                        nc.tensor.matmul(ps_q, xm[:, k, :], wqkv_sb[:, k, 0:512],
                                         start=(k == 0), stop=(k == KT - 1))
                    for k in range(KT):
                        nc.tensor.matmul(ps_kv, xm[:, k, :], wqkv_sb[:, k, 512:768],
                                         start=(k == 0), stop=(k == KT - 1))
                    # V: plain copy to [tok, d] layout (VectorE; reads PSUM)
                    nc.vector.tensor_copy(v_sb[:, m, :], ps_kv[:, 128:256])
                    col = P * m
                    # squared-sums for q0..q3|k read straight from PSUM (ScalarE)
                    sq = s12.tile([P, P], bf16, tag="sq")
                    ssum = s12.tile([P, NQ + 1], f32, tag="ssum")
                    for idx in range(NQ):
                        nc.scalar.activation(sq, ps_q[:, idx * P:(idx + 1) * P],
                                             AF.Square,
                                             accum_out=ssum[:, idx:idx + 1])
                    nc.scalar.activation(sq, ps_kv[:, 0:P], AF.Square,
                                         accum_out=ssum[:, NQ:NQ + 1])
                    rms = s12.tile([P, NQ + 1], f32, tag="rms")
                    nc.scalar.activation(rms, ssum, AF.Sqrt,
                                         bias=eps_sb[:], scale=1.0 / D)
                    rinv = s12.tile([P, NQ + 1], f32, tag="rinv")
                    nc.vector.reciprocal_approx_fast(rinv, rms)
                    # diag(1/rms) per head: folded into the transpose matmuls
                    diagr = s12.tile([P, NQ + 1, P], bf16, tag="diagr")
                    for idx in range(NQ + 1):
                        nc.vector.tensor_scalar_mul(diagr[:, idx, :], ident_sb,
                                                    rinv[:, idx:idx + 1])
                    # RoPE on raw q straight from PSUM; q heads batched via
                    # broadcast tables. (rope commutes with the per-token norm
                    # scalar, which is applied by the diag-transpose below.)
                    ro = s12.tile([P, NQ * P], bf16, tag="ro")
                    ro_v = ro.rearrange("p (h d) -> p h d", h=NQ)
                    ps_q_h = ps_q[:].rearrange("p (h d) -> p h d", h=NQ)
                    cos_b = cosq_sb[:, m, None, :].to_broadcast((P, NQ, D))
                    nc.vector.tensor_tensor(ro_v, ps_q_h, cos_b, ALU.mult)
                    rh = s12.tile([P, NQ * P], bf16, tag="rh")
                    rh_v = rh.rearrange("p (h x d) -> p h x d", h=NQ, x=2)
                    ps_q_x = ps_q[:].rearrange("p (h x d) -> p h x d", h=NQ, x=2)
                    sinq_m = sinq_sb[:, m, :].rearrange("p (x d) -> p x d", x=2)
                    sinA = sinq_m[:, None, 0, :].to_broadcast((P, NQ, 64))
                    sinB = sinq_m[:, None, 1, :].to_broadcast((P, NQ, 64))
                    nc.vector.tensor_tensor(rh_v[:, :, 0, :], ps_q_x[:, :, 1, :],
                                            sinA, ALU.mult)
                    nc.vector.tensor_tensor(rh_v[:, :, 1, :], ps_q_x[:, :, 0, :],
                                            sinB, ALU.mult)
                    nc.vector.tensor_tensor(ro, ro, rh, ALU.add)
                    # RoPE for k
                    rok = s12.tile([P, P], bf16, tag="rok")
                    nc.vector.tensor_tensor(rok, ps_kv[:, 0:P],
                                            cosk_sb[:, m, :], ALU.mult)
                    rhk = s12.tile([P, P], bf16, tag="rhk")
                    nc.vector.tensor_tensor(rhk[:, 0:64], ps_kv[:, 64:P],
                                            sink_sb[:, m, 0:64], ALU.mult)
                    nc.vector.tensor_tensor(rhk[:, 64:128], ps_kv[:, 0:64],
                                            sink_sb[:, m, 64:128], ALU.mult)
                    nc.vector.tensor_tensor(rok, rok, rhk, ALU.add)
                    if pend is not None:
                        emit_transposes(pend)
                    pend = (ro, rok, diagr, col)
                emit_transposes(pend)

            # prefetch o-projection weights during attention
            wo_sb, _wo_free = tc.tile([P, KO, HS], bf16, name="wo_sb")
            wo_src = woT.ap().rearrange("p (k f) -> p k f", k=KO)
            for k4 in range(0, KO, 4):
                nc.sync.dma_start(wo_sb[:, k4:k4 + 4, :], wo_src[:, k4:k4 + 4, :])

            # ---------------- stage 3+4: causal attention head-major; each
            # head's AllToAll overlaps the next head's compute, and its o-proj
            # pass is interleaved INTO the next head's attention blocks so the
            # PE fills the ScalarE-exp stalls (exp ~580ns/block > PE 526ns).
            acc_ctx = tc.tile_pool(name="acc", bufs=1)
            accp = acc_ctx.__enter__()
            accs = {}
            with tc.tile_pool(name="s3", bufs=6) as s3, \
                 tc.tile_pool(name="s3b", bufs=2) as s3b, \
                 tc.tile_pool(name="s4", bufs=2) as s4, \
                 tc.tile_pool(name="ps3", bufs=1, space="PSUM") as ps3, \
                 tc.tile_pool(name="ps4", bufs=1, space="PSUM") as ps4:
                a_sbs = {}

                def make_groups(h):
                    # 16 closures, each = one o-proj output tile of pass h:
                    # 4 PE matmuls (one per kv-core of the head-group) + an
                    # accumulate into the fp32 SBUF accs.
                    groups = []
                    for st in range(2):       # batch strip
                        for t2 in range(2):   # 128-tok tile within strip
                            for oc in range(OCH):
                                def g(st=st, t2=t2, oc=oc, h=h):
                                    ps_o = ps4.tile([P, 512], f32, tag="op",
                                                    bufs=3)
                                    a_sb = a_sbs[h]
                                    for gp in range(4):
                                        nc.tensor.matmul(
                                            ps_o,
                                            a_sb[:, 4 * st + gp,
                                                 P * t2:P * (t2 + 1)],
                                            wo_sb[:, 4 * gp + h,
                                                  512 * oc:512 * (oc + 1)],
                                            start=(gp == 0), stop=(gp == 3))
                                    key = (st, t2, oc)
                                    if h == 0:
                                        acc = accp.tile([P, 512], f32,
                                                        name=f"acc{st}{t2}{oc}")
                                        nc.scalar.copy(acc, ps_o)
                                        accs[key] = acc
                                    elif h < NQ - 1:
                                        nc.vector.tensor_tensor(
                                            accs[key], ps_o, accs[key], ALU.add)
                                    else:
                                        # bufs=4: the strided out-DMAs take
                                        # ~1.5us each; with 2 bufs the DVE add
                                        # chain throttles to DMA pace
                                        osb = s4.tile([P, 512], bf16,
                                                      tag="osb", bufs=4)
                                        nc.vector.tensor_tensor(
                                            osb, ps_o, accs[key], ALU.add)
                                        r0 = SW * st + P * t2
                                        nc.sync.dma_start(
                                            out_d.ap()[r0:r0 + P,
                                                       512 * oc:512 * (oc + 1)],
                                            osb)
                                groups.append(g)
                    return groups

                pend_groups = []  # (pass_id, closure) o-proj groups, FIFO
                for h in range(NQ):
                    # on the last head, interleave at most 4 groups so ~28
                    # remain to cover the final AllToAll: trigger lags the
                    # last attention block by ~6us (finalize DVE + DMA + sems)
                    # and mesh+gather is another ~17us
                    head_cap = [4 if h == NQ - 1 else 99]
                    pend_fin = None  # previous chunk's softmax finalize
                    for c in (3, 2, 1, 0):
                        qv = qT[:, h, CW * c: CW * (c + 1)]
                        nb = (c + 1) * CB
                        nd = nb - CB  # non-diagonal blocks (full width)
                        o_ps = ps3.tile([P, CW], f32, tag="o", bufs=2)
                        # exp-prob accumulator for the softmax denominator
                        pacc = s3b.tile([P, CW], bf16, tag="pacc")

                        def emit_acc(kb, pT, off, nb=nb, o_ps=o_ps):
                            # O accumulation for block kb; deferred one block so
                            # the next score matmul hides the exp latency.
                            nc.tensor.matmul(o_ps[:, off:],
                                             v_sb[:, kb, :], pT[:, off:],
                                             start=(kb == 0),
                                             stop=(kb == nb - 1))

                        pend = []
                        for kb in range(nb):
                            j = kb - nd  # diagonal index, >= 0 for diag blocks
                            off = P * j if j >= 0 else 0
                            s_ps = ps3.tile([P, CW], f32, tag="s", bufs=3)
                            nc.tensor.matmul(
                                s_ps[:, off:],
                                kT[:, P * kb: P * (kb + 1)],
                                qv[:, off:], start=True, stop=True)
                            if kb == 0:
                                pT = pacc
                            else:
                                pT = s3.tile([P, CW], bf16, tag="pT")
                            nc.scalar.activation(pT[:, off:], s_ps[:, off:],
                                                 AF.Exp)
                            if j >= 0:
                                # causal triangle on the first P columns of
                                # this diagonal block
                                nc.vector.tensor_tensor(
                                    pT[:, off:off + P], pT[:, off:off + P],
                                    tri_sb, ALU.mult)
                            # O-matmuls deferred 2 blocks so the PE never rides
                            # the exp frontier; block 0 only 1 block (its probs
                            # live in pacc, which block 1's add overwrites)
                            while pend and (pend[0][0] == 0 or len(pend) >= 2):
                                emit_acc(*pend.pop(0))
                            if kb > 0:
                                nc.vector.tensor_tensor(pacc[:, off:],
                                                        pacc[:, off:],
                                                        pT[:, off:], ALU.add)
                            pend.append((kb, pT, off))
                            if kb == 1 and pend_fin is not None:
                                # previous chunk's finalize, deferred two
                                # blocks so its DVE add-chain has drained
                                pend_fin()
                                pend_fin = None
                            # interleave pending o-proj groups into attention
                            # blocks to fill the ScalarE-exp stalls. AllToAll
                            # end-to-end latency varies 14-25us with inter-core
                            # skew, so a pass's groups are only safe TWO heads
                            # later (p <= h-2) — a too-early group hard-stalls
                            # the whole in-order PE stream. Pass NQ-2 is
                            # flushed across the final AllToAll instead, and
                            # pass NQ-1 is the tail.
                            take = (1 if (kb % 2 == 0 and c in (1, 2)) else
                                    2 if c == 0 else 0)
                            take = min(take, head_cap[0])
                            safe = h - 2
                            while take and pend_groups and \
                                    pend_groups[0][0] <= safe:
                                pend_groups.pop(0)[1]()
                                take -= 1
                                head_cap[0] -= 1
                        for p_ in pend:
                            emit_acc(*p_)

                        def finalize(c=c, pacc=pacc, o_ps=o_ps, h=h):
                            # single denominator matmul per chunk
                            sum_ps = ps3.tile([P, CW], f32, tag="s", bufs=3)
                            nc.tensor.matmul(sum_ps, onesq_sb, pacc,
                                             start=True, stop=True)
                            rec = s3b.tile([P, CW], f32, tag="rec")
                            nc.vector.reciprocal_approx_fast(rec, sum_ps)
                            o_sb = s3b.tile([P, CW], bf16, tag="o_sb")
                            nc.vector.tensor_tensor(o_sb, o_ps, rec, ALU.mult)
                            # two 256-token strips -> dest ranks 2c and 2c+1,
                            # fused into one DMA (dest rows are contiguous)
                            nc.sync.dma_start(
                                a2a_in[h][P * 2 * c:P * (2 * c + 2), :]
                                .rearrange("(blk p) t -> p blk t", p=P),
                                o_sb.rearrange("p (blk t) -> p blk t", blk=2))

                        if c > 0:
                            pend_fin = finalize
                        else:
                            finalize()
                    if h == NQ - 1:
                        # flush held-back groups: they keep the PE busy (and
                        # the HAM clock warm) across the final AllToAll
                        for _, g in pend_groups:
                            g()
                        pend_groups = []
                    nc.gpsimd.collective_compute(
                        "AllToAll", mybir.AluOpType.bypass,
                        ins=[a2a_in[h][:].opt()],
                        outs=[a2a_out[h][:].opt()],
                        replica_groups=[list(range(N_CORES))],
                    )
                    a_sb = s3.tile([P, 8, SW], bf16, tag="asb", bufs=2)
                    a2a_view = a2a_out[h][:].rearrange("(g p) t -> p g t", p=P)
                    for g8 in range(8):
                        nc.sync.dma_start(a_sb[:, g8, :], a2a_view[:, g8, :])
                    a_sbs[h] = a_sb
                    pend_groups.extend((h, g) for g in make_groups(h))
                # tail: last head's o-proj pass
                for _, g in pend_groups:
                    g()
            acc_ctx.__exit__(None, None, None)
            _wo_free()

    nc.compile()
    return nc


def shard_inputs(inputs, S=2048, HS=2048):
    """Full problem inputs -> list of 8 per-core in_maps (host-side prep)."""
    x = np.asarray(inputs["x"], np.float32)
    cos = np.asarray(inputs["cos"], np.float32)
    sin = np.asarray(inputs["sin"], np.float32)
    wq = np.asarray(inputs["wq"], np.float32)
    wk = np.asarray(inputs["wk"], np.float32)
    wv = np.asarray(inputs["wv"], np.float32)
    wo = np.asarray(inputs["wo"], np.float32)
    qw = np.asarray(inputs["q_norm_w"], np.float32)
    kw = np.asarray(inputs["k_norm_w"], np.float32)

    T = S // P
    M = 2 * T
    KT = HS // P

    xT_t = np.ascontiguousarray(
        x.reshape(M, P, KT, P).transpose(0, 3, 2, 1).reshape(M, P, HS)).astype(BF16)

    sgn = np.concatenate([-np.ones(64, np.float32), np.ones(64, np.float32)])
    scale = 1.0 / np.sqrt(D)

    def tile_p(a):
        # [(n*P), inner] row-major -> [P, n*inner] partition-major
        n = a.shape[0] // P
        return np.ascontiguousarray(
            a.reshape(n, P, a.shape[1]).transpose(1, 0, 2).reshape(P, -1))

    def fold(w, s):
        w_rot = np.concatenate([w[64:], w[:64]])
        c = tile_p((cos * w[None, :] * s).astype(np.float32)).astype(BF16)
        sn = tile_p((sin * (w_rot * sgn)[None, :] * s).astype(np.float32)).astype(BF16)
        return c, sn

    cosq, sinq = fold(qw, scale)
    cosk, sink = fold(kw, 1.0)

    r = np.arange(P)[:, None]
    t = np.arange(P)[None, :]
    tri = (r <= t).astype(BF16)

    onesq = np.ones((P, P), BF16)
    ident = np.eye(P, dtype=np.float32).astype(BF16)
    woT = tile_p(np.ascontiguousarray(wo.T)).astype(BF16)

    in_maps = []
    for c in range(N_CORES):
        b, g = c // 4, c % 4
        wq_c = wq[4 * g * D:(4 * g + 4) * D]       # [512, HS]
        wk_c = wk[g * D:(g + 1) * D]               # [128, HS]
        wv_c = wv[g * D:(g + 1) * D]               # [128, HS]
        wqkv = np.concatenate([wq_c, wk_c, wv_c], axis=0)  # [768, HS]
        wqkvT = tile_p(np.ascontiguousarray(wqkv.T)).astype(BF16)  # [P, KT*768]
        in_maps.append({
            "xT": xT_t[b * T:(b + 1) * T], "wqkvT": wqkvT, "woT": woT,
            "cosq": cosq, "sinq": sinq, "cosk": cosk, "sink": sink,
            "tri": tri, "onesq": onesq, "ident": ident,
        })
    return in_maps


def assemble(outs, S=2048, HS=2048):
    """Per-core strip outputs -> full [B, S, HS] output."""
    SW = 256
    full = np.empty((B, S, HS), np.float32)
    for c in range(N_CORES):
        o = np.asarray(outs[c], dtype=np.float32)
        full[0, c * SW:(c + 1) * SW, :] = o[0:SW]
        full[1, c * SW:(c + 1) * SW, :] = o[SW:2 * SW]
    return full


_CACHE = {}


def _get_compiled(S=2048, HS=2048):
    key = (S, HS)
    if key not in _CACHE:
        _CACHE[key] = build(S, HS)
    return _CACHE[key]


def _ensure_ntff_hook():
    """The image's antenv lacks axon_hooks; synthesize it so trace=True works."""
    import types
    try:
        from antenv.axon_hooks import get_axon_ntff_profile_hook  # noqa: F401
        return
    except ImportError:
        pass
    import antenv
    from trn_agent_boot.trn_boot import _ntff_profile_via_ctypes
    mod = types.ModuleType("antenv.axon_hooks")
    mod._hook = _ntff_profile_via_ctypes("/opt/axon/libaxon_pjrt.so")
    mod.set_axon_ntff_profile_hook = lambda h: setattr(mod, "_hook", h)
    mod.get_axon_ntff_profile_hook = lambda: mod._hook
    sys.modules["antenv.axon_hooks"] = mod
    antenv.axon_hooks = mod


def run(inputs, S=2048, HS=2048, trace=False, tmpdir=None):
    import concourse.bass_utils as bu
    if trace:
        _ensure_ntff_hook()
        bu.upload_artifacts = lambda d: ""  # no artifact bucket in this container
    nc = _get_compiled(S, HS)
    in_maps = shard_inputs(inputs, S, HS)
    res = bu.run_bass_kernel_spmd(nc, in_maps, core_ids=list(range(N_CORES)),
                                  trace=trace, tmpdir=tmpdir)
    out = assemble([r["out"] for r in res.results], S, HS)
    return out, res.exec_time_ns


def kernel(**inputs):
    out, _ = run(inputs)
    return out
